# revision 26
# baseline (speedup 1.0000x reference)
"""Trainium2 Bass kernel for Felsenstein pruning on a perfect binary tree
(nn_BaseLikelihoodModel: batched expm over edges + level-synchronous sweep).

SINGLE-CORE variant: the whole 65536-node tree runs on core 0.
Rationale: the mid-kernel AllGather of the 8-core version costs ~15us fixed
(mesh algo + trigger + post-DMA) plus full exposure to PJRT launch stagger
(core 0 launches first and waits 10..40us for the last peer).  One core
does 8x the arithmetic but the kernel is latency/overhead-bound, not
throughput-bound: measured multi-core 79-96us vs ~50us estimated here,
deterministically.

Math (same as the multi-core version):
  expm(t R) v = W * B(t) * (W^-1 v) for R = Q - diag(growth), via a real
  block eigendecomposition host-side.  swap-fold: Winv[swap] = P Winv and
  pairs share Re(lambda) with opposite Im signs, so
      Y = W (cos * EU) + W[:,swap] ((-sin) * EU),   EU = E * (Winv V),
  with E = exp(a t + BOOST) from the Scalar exp table, cos/-sin as 2-term
  polynomials in x = b t (|x| <= 0.35), everything bf16, inputs fp16.
  E is folded at the per-level EU stage so the edge-factor precompute is
  only 3 elementwise passes (cos, q, -sin) instead of 5.

Layout: 8 partition-blocks x 16 states; block u owns leaves
[4096u, 4096(u+1)).  Levels 1..12 sweep in-block (level h width 8192>>h
columns, chunked by 512 for PSUM banks at h<=4); levels 13..16 run on a
stacked 32-partition single block.  One per-node rescale at level 8
(accumulated log-scales).  Edge factors carry e^BOOST; (N-1)*BOOST is
subtracted exactly at the end.
"""
import math
import numpy as np
import ml_dtypes

import concourse.bass as bass
import concourse.mybir as mybir
import concourse.tile as tile
from concourse.bass_utils import run_bass_kernel_spmd

F32 = np.float32
F16 = np.float16
BF16 = ml_dtypes.bfloat16
S = 16
L = 32768
N = 2 * L
LPB = 4096                 # leaves per partition-block
BLK_W = [LPB >> hc for hc in range(12)]            # 4096..2
BLK_OFF = np.concatenate([[0], np.cumsum(BLK_W)])  # offsets into 8190
TOPO = int(BLK_OFF[-1])                            # 8190
T_ALL = TOPO + 15                                  # + [lvl12 x8][13 x4][14 x2][15 x1]
NARROW_LO = int(BLK_OFF[4])                        # 7680: levels 5..12 + top
NARROW_W = T_ALL - NARROW_LO                       # 525
NPAD = 528                                         # stride for [EC|ESp] narrow tile
CHUNKS = [(i * 512, min(512, T_ALL - i * 512)) for i in range((T_ALL + 511) // 512)]

OFFS = [0]
for _h in range(1, 16):
    OFFS.append(OFFS[-1] + (L >> (_h - 1)))

BOOST = 1.7
CORR = float(np.float64(N - 1) * np.float64(np.float32(BOOST)))

# pack16 column layout (fp16 [8, P16_COLS])
P16_EXPD = 0                   # kron(I8, ones(1,16))
P16_EXPB = 128                 # kron(I8, bsig row)
P16_T = 256                    # t_blk
P16_SID = 256 + T_ALL          # leaf state ids [8, 4096]
P16_COLS = 256 + T_ALL + LPB
# packbf column layout (bf16 [128, 584])
PB_WINV = 0
PB_W = 128
PB_WSW = 256
PB_ONESBD = 384
PB_ONESC = 512
PB_ITILE = 520
PB_U2 = 536      # [Winv.T | Winv.T]   [0:16, 536:568]
PB_Y2 = 568      # [W.T ; Wsw.T]       [0:32, 568:584]
PB_COLS = 584
# packf column layout (f32 [128, 20])
PF_AVEC = 0
PF_BVEC = 1
PF_IOTA = 2
PF_GCOL = 3
PF_ONESF = 4     # ones row-0 [0:1, 4:20]
PF_COLS = 20


def _real_eig(R):
    """Real block eigendecomposition R = Wr @ M @ inv(Wr)."""
    ev, V = np.linalg.eig(R)
    used = np.zeros(S, bool)
    order = np.argsort(-ev.real)
    cols = []
    for i in order:
        if used[i]:
            continue
        if abs(ev[i].imag) < 1e-12:
            cols.append(("real", i))
            used[i] = True
        else:
            j = None
            for i2 in order:
                if not used[i2] and i2 != i and abs(ev[i2] - ev[i].conj()) < 1e-8:
                    j = i2
                    break
            assert j is not None, "unpaired complex eigenvalue"
            ip = i if ev[i].imag > 0 else j
            cols.append(("pair", ip))
            used[i] = used[j] = True
    Wr = np.zeros((S, S))
    a = np.zeros(S)
    bsig = np.zeros(S)
    swap = np.arange(S)
    k = 0
    for c in cols:
        if c[0] == "real":
            i = c[1]
            Wr[:, k] = V[:, i].real
            a[k] = ev[i].real
            k += 1
        else:
            ip = c[1]
            lam = ev[ip]
            Wr[:, k] = V[:, ip].real
            Wr[:, k + 1] = V[:, ip].imag
            a[k] = a[k + 1] = lam.real
            bsig[k] = lam.imag
            bsig[k + 1] = -lam.imag
            swap[k] = k + 1
            swap[k + 1] = k
            k += 2
    assert k == S
    scales = np.ones(S)
    kk = 0
    while kk < S:
        if swap[kk] == kk:
            scales[kk] = np.linalg.norm(Wr[:, kk])
            kk += 1
        else:
            s = math.sqrt(np.linalg.norm(Wr[:, kk]) * np.linalg.norm(Wr[:, kk + 1]))
            scales[kk] = scales[kk + 1] = s
            kk += 2
    Wr = Wr / scales[None, :]
    Winv = np.linalg.inv(Wr)
    return Wr, Winv, a, bsig, swap


def _split_multi_waits(nc):
    """Walrus allows ONE sync-wait slot per engine instruction; move extras
    onto prepended same-engine NoOps."""
    skip = (mybir.InstAllEngineBarrier, mybir.InstBranchHint,
            mybir.InstCompareAndBranch, mybir.InstUnconditionalBranch,
            mybir.InstIndirectBranch)
    for fn in nc.m.functions:
        for blk in fn.blocks:
            out = []
            for inst in blk.instructions:
                si = inst.sync_info
                if (si is not None and si.on_wait and len(si.on_wait) > 1
                        and not isinstance(inst, skip)):
                    waits = list(si.on_wait)
                    for i, w in enumerate(waits[:-1]):
                        nop = mybir.InstNoOp(
                            name=f"{inst.name}-wait{i}", ins=[], outs=[])
                        nop.engine = inst.engine
                        nop.sync_info = mybir.SyncInfo(
                            on_wait=[w], on_update=[])
                        out.append(nop)
                    inst.sync_info = mybir.SyncInfo(
                        on_wait=[waits[-1]], on_update=list(si.on_update or []))
                out.append(inst)
            blk.instructions = out


def build_nc(split_waits=True):
    f32 = mybir.dt.float32
    bf16 = mybir.dt.bfloat16
    f16 = mybir.dt.float16
    AF = mybir.ActivationFunctionType
    OP = mybir.AluOpType
    nc = bass.Bass()

    pack16 = nc.dram_tensor("pack16", [8, P16_COLS], f16, kind="ExternalInput")
    t128d = nc.dram_tensor("t128d", [128, T_ALL], f16, kind="ExternalInput")
    sid128d = nc.dram_tensor("sid128d", [128, LPB], f16, kind="ExternalInput")
    packbf = nc.dram_tensor("packbf", [128, PB_COLS], bf16, kind="ExternalInput")
    packf = nc.dram_tensor("packf", [128, PF_COLS], f32, kind="ExternalInput")
    out = nc.dram_tensor("out", [16, 1], f32, kind="ExternalOutput")

    with tile.TileContext(nc) as tc:
        with (
            tc.tile_pool(name="const", bufs=1) as cp,
            tc.tile_pool(name="sb", bufs=2) as sb,
            tc.tile_pool(name="big", bufs=1) as bigp,
            tc.tile_pool(name="psA", bufs=2, space="PSUM") as psA,
            tc.tile_pool(name="psU", bufs=2, space="PSUM") as psU,
            tc.tile_pool(name="psY", bufs=2, space="PSUM") as psY,
        ):
            s_p16 = cp.tile([8, P16_COLS], f16, tag="p16")
            nc.scalar.dma_start(s_p16[:], pack16[:, :])
            T128 = bigp.tile([128, T_ALL], f16, tag="T128")
            TQ = (T_ALL + 3) // 4
            for _q in range(4):
                _lo = _q * TQ
                _w = min(TQ, T_ALL - _lo)
                eng = (nc.scalar, nc.sync, nc.gpsimd, nc.scalar)[_q]
                eng.dma_start(T128[:, _lo:_lo + _w], t128d[:, _lo:_lo + _w])
            SID128 = bigp.tile([128, LPB], f16, tag="SID128")
            nc.sync.dma_start(SID128[:, 0:2048], sid128d[:, 0:2048])
            nc.gpsimd.dma_start(SID128[:, 2048:4096], sid128d[:, 2048:4096])
            s_pf = cp.tile([128, PF_COLS], f32, tag="packf")
            nc.gpsimd.dma_start(s_pf[:], packf[:, :])
            s_pb = cp.tile([128, PB_COLS], bf16, tag="packbf")
            nc.sync.dma_start(s_pb[:], packbf[:, :])

            v_expd = s_p16[:, P16_EXPD:P16_EXPD + 128]
            v_expB = s_p16[:, P16_EXPB:P16_EXPB + 128]
            v_t = s_p16[:, P16_T:P16_T + T_ALL]
            v_sid = s_p16[:, P16_SID:P16_SID + LPB]
            c_winvT = s_pb[:, PB_WINV:PB_WINV + 128]
            c_wT = s_pb[:, PB_W:PB_W + 128]
            c_wswT = s_pb[:, PB_WSW:PB_WSW + 128]
            c_onesbd = s_pb[:, PB_ONESBD:PB_ONESBD + 128]
            c_onesc = s_pb[:, PB_ONESC:PB_ONESC + 8]
            c_itile = s_pb[:, PB_ITILE:PB_ITILE + 16]
            c_u2T = s_pb[0:16, PB_U2:PB_U2 + 32]
            c_y2T = s_pb[0:32, PB_Y2:PB_Y2 + 16]
            c_avec = s_pf[:, PF_AVEC:PF_AVEC + 1]
            c_bvec = s_pf[:, PF_BVEC:PF_BVEC + 1]
            c_iota = s_pf[:, PF_IOTA:PF_IOTA + 1]
            c_gcol = s_pf[:, PF_GCOL:PF_GCOL + 1]
            c_onesf = s_pf[0:1, PF_ONESF:PF_ONESF + 16]

            cBOOST = cp.tile([128, 1], f32, tag="boost")
            nc.vector.memset(cBOOST[:], float(BOOST))
            ones8 = cp.tile([8, 1], f32, tag="ones8")
            nc.vector.memset(ones8[:], 1.0)
            # dummy activation: pull the exp+ln ACT table load to boot time
            dummy = cp.tile([1, 1], f32, tag="dummy")
            nc.scalar.activation(dummy[:], cBOOST[0:1, 0:1], AF.Exp)

            # PE queue observers (one tiny matmul per DMA'd matmul operand)
            pobs = psY.tile([1, 1], f32, tag="Ye")
            nc.tensor.matmul(pobs[:], s_p16[0:1, 0:1], s_p16[0:1, 0:1],
                             start=True, stop=True)
            pobs2 = psY.tile([1, 1], f32, tag="Yo")
            nc.tensor.matmul(pobs2[:], s_pb[0:1, 0:1], s_pb[0:1, 0:1],
                             start=True, stop=True)
            pobs3 = psU.tile([1, 1], f32, tag="U")
            nc.tensor.matmul(pobs3[:], s_pf[0:1, 0:1], s_pf[0:1, 0:1],
                             start=True, stop=True)

            # ---- leaf one-hots: direct compare of pre-broadcast state ids
            sX = bigp.tile([128, LPB], bf16, tag="V0")
            for k in range(4):
                nc.vector.tensor_scalar(sX[:, 1024 * k:1024 * (k + 1)],
                                        SID128[:, 1024 * k:1024 * (k + 1)],
                                        c_iota, None, OP.is_equal)

            # ---- edge factors: E = exp(a t + BOOST); CN = [cos | -sin] of
            # x = b t (2-term polys); E is folded later at the EU stage.
            sE = bigp.tile([128, T_ALL], bf16, tag="sE")
            CN = bigp.tile([128, 2 * T_ALL], bf16, tag="CN")
            for lo, wch in CHUNKS:
                ts_ = T128[:, lo:lo + wch]
                nc.scalar.activation(sE[:, lo:lo + wch], ts_, AF.Exp,
                                     bias=cBOOST[:, 0:1], scale=c_avec)
                x2 = sb.tile([128, wch], bf16, tag="x2")
                nc.scalar.activation(x2[:], ts_, AF.Square, scale=c_bvec)
                nc.vector.tensor_scalar(CN[:, lo:lo + wch], x2[:], -0.5, 1.0,
                                        OP.mult, OP.add)
                q = sb.tile([128, wch], bf16, tag="q")
                nc.vector.tensor_scalar(q[:], x2[:], float(1.0 / 6.0), -1.0,
                                        OP.mult, OP.add)
                # -sin = (b t) * q in one stt pass straight from T128
                nc.vector.scalar_tensor_tensor(
                    CN[:, T_ALL + lo:T_ALL + lo + wch], ts_, c_bvec, q[:],
                    OP.mult, OP.mult)

            # narrow-level factors with E pre-multiplied: [EC | ESp] packed
            ECESn = bigp.tile([128, 2 * NPAD], bf16, tag="ECESn")
            nc.vector.tensor_mul(ECESn[:, 0:NARROW_W],
                                 CN[:, NARROW_LO:T_ALL],
                                 sE[:, NARROW_LO:T_ALL])
            nc.gpsimd.tensor_mul(ECESn[:, NPAD:NPAD + NARROW_W],
                                 CN[:, T_ALL + NARROW_LO:2 * T_ALL],
                                 sE[:, NARROW_LO:T_ALL])
            ecs3n = ECESn[:].rearrange("p (k w) -> p k w", k=2)

            # stacked top-edge factors for levels 13..16 (SBUF->SBUF DMA
            # sidesteps the 32-aligned partition-base rule)
            TOPN = TOPO - NARROW_LO  # 510
            ECS = sb.tile([32, 15], bf16, tag="ECS")
            nc.sync.dma_start(ECS[0:16, :], ECESn[0:16, TOPN:TOPN + 15])
            nc.sync.dma_start(ECS[16:32, :],
                              ECESn[0:16, NPAD + TOPN:NPAD + TOPN + 15])

            # ---- wide levels 1..4, column-chunked by 512: each chunk of 512
            # children yields 256 parents independently.
            V = sX
            for h in range(1, 5):
                wc = BLK_W[h - 1]
                lo = int(BLK_OFF[h - 1])
                nch = wc // 512
                Vn = bigp.tile([128, wc // 2], bf16, tag=f"V{h}")
                for k in range(nch):
                    cl = 512 * k
                    pU = psU.tile([128, 512], f32, tag="U")
                    nc.tensor.matmul(pU[:], c_winvT, V[:, cl:cl + 512],
                                     start=True, stop=True)
                    EU = sb.tile([128, 512], bf16, tag="EU")
                    nc.vector.tensor_mul(EU[:], sE[:, lo + cl:lo + cl + 512],
                                         pU[:])
                    m1 = sb.tile([128, 512], bf16, tag="m1")
                    nc.vector.tensor_mul(m1[:], CN[:, lo + cl:lo + cl + 512],
                                         EU[:])
                    m2 = sb.tile([128, 512], bf16, tag="m2")
                    nc.gpsimd.tensor_mul(
                        m2[:], CN[:, T_ALL + lo + cl:T_ALL + lo + cl + 512],
                        EU[:])
                    pYe = psY.tile([128, 256], f32, tag="Ye")
                    nc.tensor.matmul(pYe[:], c_wT, m1[:, 0::2],
                                     start=True, stop=False)
                    nc.tensor.matmul(pYe[:], c_wswT, m2[:, 0::2],
                                     start=False, stop=True)
                    sYe = sb.tile([128, 256], f32, tag="sYe")
                    nc.scalar.copy(sYe[:], pYe[:])
                    pYo = psY.tile([128, 256], f32, tag="Yo")
                    nc.tensor.matmul(pYo[:], c_wT, m1[:, 1::2],
                                     start=True, stop=False)
                    nc.tensor.matmul(pYo[:], c_wswT, m2[:, 1::2],
                                     start=False, stop=True)
                    nc.vector.scalar_tensor_tensor(
                        Vn[:, 256 * k:256 * (k + 1)], pYo[:], c_gcol, sYe[:],
                        OP.mult, OP.mult)
                V = Vn

            # ---- narrow levels 5..12: merged m12 = [EC|ESp] * [U|U]
            lsW = None
            for h in range(5, 13):
                wc = BLK_W[h - 1]
                lo = int(BLK_OFF[h - 1]) - NARROW_LO
                wp = wc // 2
                pU = psU.tile([128, 2 * wc], f32, tag="U")
                nc.tensor.matmul(pU[:, 0:wc], c_winvT, V[:], start=True, stop=True)
                nc.tensor.matmul(pU[:, wc:2 * wc], c_winvT, V[:],
                                 start=True, stop=True)
                # m12 = [EC|ESp] * [U|U]; E is already inside ECESn here
                m12 = sb.tile([128, 2 * wc], bf16, tag="m12")
                nc.vector.tensor_mul(
                    m12[:].rearrange("p (k w) -> p k w", k=2),
                    ecs3n[:, :, lo:lo + wc],
                    pU[:].rearrange("p (k w) -> p k w", k=2))
                pYe = psY.tile([128, wp], f32, tag="Ye")
                nc.tensor.matmul(pYe[:], c_wT, m12[:, 0:wc:2],
                                 start=True, stop=False)
                nc.tensor.matmul(pYe[:], c_wswT, m12[:, wc:2 * wc:2],
                                 start=False, stop=True)
                sYe = sb.tile([128, wp], f32, tag="sYe")
                nc.vector.tensor_copy(sYe[:], pYe[:])
                pYo = psY.tile([128, wp], f32, tag="Yo")
                nc.tensor.matmul(pYo[:], c_wT, m12[:, 1:wc:2],
                                 start=True, stop=False)
                nc.tensor.matmul(pYo[:], c_wswT, m12[:, wc + 1:2 * wc:2],
                                 start=False, stop=True)
                if h == 8:
                    praw = sb.tile([128, wp], bf16, tag="Vc")
                    nc.vector.scalar_tensor_tensor(
                        praw[:], pYo[:], c_gcol, sYe[:], OP.mult, OP.mult)
                    pSb = psA.tile([128, wp], f32, tag="T")
                    nc.tensor.matmul(pSb[:], c_onesbd, praw[:],
                                     start=True, stop=True)
                    pSc = psU.tile([8, wp], f32, tag="U")
                    nc.tensor.matmul(pSc[:], c_onesc, praw[:],
                                     start=True, stop=True)
                    rb = sb.tile([128, wp], f32, tag="rb")
                    nc.vector.reciprocal(rb[:], pSb[:])
                    Vn = sb.tile([128, wp], bf16, tag="V")
                    nc.vector.tensor_mul(Vn[:], praw[:], rb[:])
                    lnS = sb.tile([8, wp], f32, tag="lnS")
                    nc.scalar.activation(lnS[:], pSc[:], AF.Ln)
                    lsW = lnS
                else:
                    Vn = sb.tile([128, wp], f32 if h == 12 else bf16, tag="V")
                    nc.vector.scalar_tensor_tensor(
                        Vn[:], pYo[:], c_gcol, sYe[:], OP.mult, OP.mult)
                V = Vn

            # log-scale total: sum lsW [8,8] -> scalar
            lsr = sb.tile([8, 1], f32, tag="lsr")
            nc.vector.tensor_reduce(lsr[:], lsW[:], mybir.AxisListType.X, OP.add)
            pls = psU.tile([1, 1], f32, tag="U")
            nc.tensor.matmul(pls[:], lsr[:], ones8[:], start=True, stop=True)
            tot = sb.tile([1, 1], f32, tag="tot")
            nc.vector.tensor_scalar_add(tot[:], pls[:], float(-CORR))

            # ---- reshape block roots: V (128x1) -> (16x8)
            rhs8 = sb.tile([128, 8], bf16, tag="rhs8")
            nc.vector.tensor_scalar_mul(rhs8[:], c_onesc, V[:, 0:1])
            pV9 = psY.tile([16, 8], f32, tag="Ye")
            nc.tensor.matmul(pV9[:], c_itile, rhs8[:], start=True, stop=True)
            sV = sb.tile([16, 8], bf16, tag="sV")
            nc.vector.tensor_copy(sV[:], pV9[:])

            # ---- levels 13..16 stacked on 32 partitions
            def top_level(rhsV, off, n):
                pU2 = psU.tile([32, n], f32, tag="U")
                nc.tensor.matmul(pU2[:], c_u2T, rhsV, start=True, stop=True)
                m12 = sb.tile([32, n], bf16, tag="m12t")
                nc.vector.tensor_mul(m12[:], ECS[:, off:off + n], pU2[:])
                if n == 1:
                    pYt = psY.tile([16, 1], f32, tag="Ye")
                    nc.tensor.matmul(pYt[:], c_y2T, m12[:], start=True, stop=True)
                    return pYt, None
                pYe = psY.tile([16, n // 2], f32, tag="Ye")
                nc.tensor.matmul(pYe[:], c_y2T, m12[:, 0::2], start=True, stop=True)
                sYe = sb.tile([16, n // 2], f32, tag="sYe")
                nc.vector.tensor_copy(sYe[:], pYe[:])
                pYo = psY.tile([16, n // 2], f32, tag="Yo")
                nc.tensor.matmul(pYo[:], c_y2T, m12[:, 1::2], start=True, stop=True)
                return pYo, sYe

            off = 0
            n = 8
            for h in (13, 14, 15):
                pYo, sYe = top_level(sV[:], off, n)
                off += n
                n //= 2
                sV = sb.tile([16, n], bf16, tag="sV")
                nc.vector.scalar_tensor_tensor(
                    sV[:], pYo[:], c_gcol[0:16, 0:1], sYe[:], OP.mult, OP.mult)
            # root: unifurcating, left child only, no growth
            pYt, _ = top_level(sV[:], 14, 1)

            lnv = sb.tile([16, 1], f32, tag="lnv")
            nc.scalar.activation(lnv[:], pYt[:], AF.Ln)
            ptb = psU.tile([16, 1], f32, tag="U")
            nc.tensor.matmul(ptb[:], c_onesf, tot[:], start=True, stop=True)
            outv = sb.tile([16, 1], f32, tag="outv")
            nc.vector.tensor_add(outv[:], lnv[:], ptb[:])
            nc.sync.dma_start(out[:, :], outv[:])

    if split_waits:
        _split_multi_waits(nc)
    return nc


def _host_prep(branch_lens, init_partials, Q, growth_rates):
    bl = np.ascontiguousarray(np.asarray(branch_lens, dtype=F32))
    ip = np.asarray(init_partials, dtype=F32)
    Q64 = np.asarray(Q, dtype=np.float64)
    g64 = np.asarray(growth_rates, dtype=np.float64)
    R = Q64 - np.diag(g64)
    Wr, Winv, a, bsig, swap = _real_eig(R)
    Wsw = Wr[:, swap]

    I8 = np.eye(8)

    def bf(x):
        return np.asarray(x, dtype=np.float32).astype(BF16)

    packbf = np.zeros((128, PB_COLS), dtype=BF16)
    packbf[:, PB_WINV:PB_WINV + 128] = bf(np.kron(I8, Winv.T))
    packbf[:, PB_W:PB_W + 128] = bf(np.kron(I8, Wr.T))
    packbf[:, PB_WSW:PB_WSW + 128] = bf(np.kron(I8, Wsw.T))
    packbf[:, PB_ONESBD:PB_ONESBD + 128] = bf(np.kron(I8, np.ones((S, S))))
    packbf[:, PB_ONESC:PB_ONESC + 8] = bf(np.kron(I8, np.ones((S, 1))))
    packbf[:, PB_ITILE:PB_ITILE + 16] = bf(np.tile(np.eye(S), (8, 1)))
    packbf[0:16, PB_U2:PB_U2 + 32] = bf(np.hstack([Winv.T, Winv.T]))
    packbf[0:32, PB_Y2:PB_Y2 + 16] = bf(np.vstack([Wr.T, Wsw.T]))

    packf = np.zeros((128, PF_COLS), dtype=F32)
    packf[:, PF_AVEC] = np.tile(a, 8)
    packf[:, PF_BVEC] = np.tile(bsig, 8)
    packf[:, PF_IOTA] = np.arange(128) % 16
    packf[:, PF_GCOL] = np.tile(g64, 8)
    packf[0, PF_ONESF:PF_ONESF + 16] = 1.0

    states = np.argmax(ip[:L], axis=1).astype(F32)

    t_blk = np.zeros((8, T_ALL), dtype=F32)
    for hc in range(12):
        w = LPB >> hc
        seg = bl[OFFS[hc]: OFFS[hc] + (L >> hc)].reshape(8, w)
        t_blk[:, int(BLK_OFF[hc]): int(BLK_OFF[hc]) + w] = seg
    tt = np.concatenate([
        bl[OFFS[12]: OFFS[12] + 8],
        bl[OFFS[13]: OFFS[13] + 4],
        bl[OFFS[14]: OFFS[14] + 2],
        bl[OFFS[15]: OFFS[15] + 1],
    ])
    t_blk[0, TOPO:TOPO + 15] = tt
    t_blk[1, TOPO:TOPO + 15] = tt

    p16 = np.zeros((8, P16_COLS), dtype=F16)
    p16[:, P16_EXPD:P16_EXPD + 128] = np.kron(I8, np.ones((1, S)))
    p16[:, P16_EXPB:P16_EXPB + 128] = np.kron(I8, bsig[None, :])
    p16[:, P16_T:P16_T + T_ALL] = t_blk.astype(F16)
    p16[:, P16_SID:P16_SID + LPB] = states.reshape(8, LPB).astype(F16)

    t128 = np.ascontiguousarray(np.repeat(t_blk.astype(F16), 16, axis=0))
    sid128 = np.ascontiguousarray(
        np.repeat(states.reshape(8, LPB).astype(F16), 16, axis=0))
    return [{"pack16": np.ascontiguousarray(p16),
             "packbf": np.ascontiguousarray(packbf),
             "packf": np.ascontiguousarray(packf),
             "t128d": t128, "sid128d": sid128}]


def kernel(postorder, children, parents, branch_lens, init_partials, Q,
           levels, growth_rates, *, _trace=False):
    in_maps = _host_prep(branch_lens, init_partials, Q, growth_rates)
    nc = build_nc()
    res = run_bass_kernel_spmd(nc, in_maps, core_ids=[0], trace=_trace)
    out = np.asarray(res.results[0]["out"], dtype=F32).reshape(S)
    if _trace:
        kernel.last_exec_time_ns = res.exec_time_ns
        kernel.last_results = res
    return out
